# revision 28
# baseline (speedup 1.0000x reference)
"""Trainium2 Bass kernel for a 3-branch GCN layer (sum of three GCNConvs).

Math: out[b,t] = sum_k A_k @ (x[b,t] @ W_k) + b_k, with A_k the normalized
adjacency (self loops) of the k-th tiny 25-node graph shared across (B,T).

Instead of the dense [1600x1600] kron operator (one big GEMM, ~395k PE
row-cycles/core), factor into two chained PE stages with NO on-chip
transposes (host pre-transposes x, which is free):

  stage W:  Y[btn, (k,c)] = X[btn, c'] @ [W1|W2|W3]      (K=64, F=192)
  stage A:  out[btn, c]   = sum_k kron(I5, A_k^T) @ Y_k  (K=125, F=64 x3)

Tiles are 125 rows = 5 (b,t) groups x 25 nodes, padded to 128 so every
stationary is an aligned [128,128] load. Key device-side economics
(measured on TRN2): matmuls carry a ~173ns fixed latency and ~100ns
LDWEIGHTS, so everything is batched fat:

 - W-stage: TWO tiles per matmul: lhsT stacks both tiles' channels in
   K=128, rhs is block-diag [[Wcat,0],[0,Wcat]] -> F=384, 240 matmuls.
 - A-stage: 3 matmuls of F=512 per 8-tile super-group via a 3D rhs over
   the k-major ysb [128, 3, 8t, 64c]; out fills a whole psum bank.
 - PSUM = one 8-bank ring tile: banks 0-5 hold pair-Y accumulations,
   banks 6-7 double-buffer the super-group outputs (accumulation state
   is bank-granular, so exactly one accumulation group per bank).
 - Y is cast fp32->fp16 by per-pair copies alternating ACT/DVE (the only
   PSUM-capable engines), with the (t,k,c)->(k,t,c) transpose folded
   into the copy's AP walk (64-elem contiguous runs).
 - out: fp16 staging copies + gpsimd-queue DMAs; host un-permutes.

Data-parallel over batch: 8 batches (2400 bt rows) per core x 8 cores.
Bias is added on the host (typically zero; np.any fast-path).
"""

import sys

import numpy as np

if "/opt/trn_rl_repo" not in sys.path:
    sys.path.insert(0, "/opt/trn_rl_repo")

B, T, NNODES, C = 64, 300, 25, 64
N_CORES = 8
BT_LOC = (B // N_CORES) * T          # 2400 (b,t) rows per core
ROWS_LOC = BT_LOC * NNODES           # 60000 btn rows per core
TILE = 125                           # 5 bt-groups x 25 nodes
NTILES = ROWS_LOC // TILE            # 480
NGRP = NTILES // 4                   # 120 groups of 4 tiles
NCHUNK = 8                           # x input DMA chunks
HALF = NTILES // 2                   # tiles per partition-half (240)
CHW = NTILES // 2 * 128 // NCHUNK    # x chunk width in elements (3840)
BANKC = 512                          # fp32 elems per psum bank partition-row
OOFF = 256                           # out region offset within a bank

_PROGRAM_CACHE = {}
# extra kwargs for run_bass_kernel_spmd (test harness sets trace=True here)
_RUN_KW = {}


def _dense_adj(edge_index_k: np.ndarray) -> np.ndarray:
    """PyG GCNConv normalized dense adjacency A[dst, src] (float64)."""
    row = edge_index_k[0].astype(np.int64)
    col = edge_index_k[1].astype(np.int64)
    loop = np.arange(NNODES, dtype=np.int64)
    row = np.concatenate([row, loop])
    col = np.concatenate([col, loop])
    deg = np.zeros(NNODES, dtype=np.float64)
    np.add.at(deg, col, 1.0)
    dinv = np.where(deg > 0, 1.0 / np.sqrt(deg), 0.0)
    norm = dinv[row] * dinv[col]
    A = np.zeros((NNODES, NNODES), dtype=np.float64)
    np.add.at(A, (col, row), norm)
    return A


def _build_program():
    import concourse.bass as bass
    import concourse.tile as tile
    from concourse import bacc, mybir

    f32 = mybir.dt.float32
    f16 = mybir.dt.float16

    nc = bacc.Bacc(
        "TRN2", target_bir_lowering=False, debug=False, num_devices=N_CORES
    )
    # host-pretransposed x, tile-pair stacked: [128, 240*128] fp16; rows
    # 0-63 = channels of even tile, 64-127 = channels of odd tile
    xh = nc.dram_tensor(
        "xh", [128, NTILES // 2 * 128], f16, kind="ExternalInput"
    ).ap()
    # block-diagonal [[Wcat, 0], [0, Wcat]] for pair-fused W-matmuls
    wh = nc.dram_tensor("wh", [128, 6 * C], f16, kind="ExternalInput").ap()
    # three zero-padded block-diagonal stationaries kron(I5, A_k^T): [3,128,128]
    ah = nc.dram_tensor("ah", [3, 128, 128], f16, kind="ExternalInput").ap()
    # permuted output: dev[p, m, t, c] = out for btn tile 8m+t, row p, chan c
    dev = nc.dram_tensor(
        "dev", [TILE, NTILES // 8, 8, C], f16, kind="ExternalOutput"
    ).ap()

    DEPTH = 2  # software-pipeline distance, in 8-tile super-groups

    with tile.TileContext(nc) as tc:
        with (
            tc.tile_pool(name="const", bufs=1) as const_pool,
            tc.tile_pool(name="ysb", bufs=6) as ysb_pool,
            tc.tile_pool(name="ostg", bufs=3) as ostg_pool,
            tc.tile_pool(name="ring", bufs=1, space="PSUM") as ring_pool,
        ):
            # the whole of PSUM as one 8-bank ring
            big = ring_pool.tile([128, 8, BANKC], f32, tag="ring", name="ring")

            # constants on the scalar HWDGE queue
            wsb = const_pool.tile([128, 6 * C], f16, tag="wcat")
            nc.scalar.dma_start(wsb[:], wh[:])
            asb = []
            for k in range(3):
                t = const_pool.tile([128, 128], f16, tag=f"a{k}")
                nc.scalar.dma_start(t[:], ah[k])
                asb.append(t)
            
            # x streamed in variable chunks on the sync (SP) queue; the
            # first chunk is small so the PE starts sooner
            sizes = [8, 22] + [30] * 7  # pairs per chunk (240 total)
            xsb = []
            pair_loc = {}
            p0 = 0
            for ci, sz in enumerate(sizes):
                t = const_pool.tile(
                    [128, sz * 128], f16, tag=f"x{ci}", name=f"x{ci}"
                )
                nc.sync.dma_start(
                    t[:], xh[:, p0 * 128 : (p0 + sz) * 128]
                )
                xsb.append(t)
                for j in range(sz):
                    pair_loc[p0 + j] = (ci, j * 128)
                p0 += sz

            def xpair(p):
                # lhsT [128, 128] for tile pair p = tiles (2p, 2p+1)
                ci, off = pair_loc[p]
                return xsb[ci][0:128, off : off + 128]

            ysbs = {}

            def copy(n, dst, src):
                # only DVE and ACT can read PSUM
                if n % 2 == 0:
                    nc.scalar.copy(dst, src)
                else:
                    nc.vector.tensor_copy(dst, src)

            ncopies = [0]
            NSG = NTILES // 8  # 60 super-groups of 8 tiles

            OG = 2  # super-groups per out staging buffer / DMA
            ostg = {"t": None}

            def emit_a_mm(m, k):
                # one accumulating A-matmul, F=512, out = full bank 6 or 7
                ysb = ysbs[m]
                ob = 6 + (m % 2)
                nc.tensor.matmul(
                    big[0:128, ob, 0:BANKC],
                    asb[k][:],
                    ysb[0:128, k, :, :],
                    start=(k == 0), stop=(k == 2),
                )

            def emit_a_out(m):
                # drain BOTH out banks (super-groups m-1, m) in one copy
                ysbs.pop(m)
                if m % 2 == 0:
                    return
                st = ostg_pool.tile([TILE, 2, BANKC], f16, tag="ostg", name="ostg")
                copy(ncopies[0], st[:], big[0:TILE, 6:8, 0:BANKC])
                ncopies[0] += 1
                nc.gpsimd.dma_start(dev[:, m - 1 : m + 1, :, :], st[:])

            def emit_w_mm(p):
                s = p % 6
                nc.tensor.matmul(
                    big[0:128, s, 0 : 6 * C],
                    xpair(p), wsb[:],
                    start=True, stop=True,
                )

            def emit_y_copy(m, u, ysb):
                # one copy drains TWO pair-banks: walk (bank, t'', k, c) on
                # the source == (t, k, c) on the k-major ysb destination
                s = (4 * m + 2 * u) % 6
                dst = ysb[0:128, :, 4 * u : 4 * u + 4, :].rearrange(
                    "p k t c -> p t k c"
                )
                copy(
                    ncopies[0],
                    dst,
                    big[0:128, s : s + 2, 0 : 6 * C],
                )
                ncopies[0] += 1

            def emit_sg(m):
                # W-pairs with the previous super-group's fat A-matmuls
                # interleaved between them (v4 ordering)
                ysb = None
                if m < NSG:
                    ysb = ysb_pool.tile([128, 3, 8, C], f16, tag="y", name="y")
                    ysbs[m] = ysb
                for h in range(4):
                    if m < NSG:
                        emit_w_mm(4 * m + h)
                    if m >= DEPTH and h < 3:
                        emit_a_mm(m - DEPTH, h)
                    if m < NSG and h % 2 == 1:
                        emit_y_copy(m, h // 2, ysb)
                if m >= DEPTH:
                    emit_a_out(m - DEPTH)

            for m in range(NSG + DEPTH):
                emit_sg(m)

    nc.compile()
    return nc


def kernel(x, edge_index, W1, W2, W3, b1, b2, b3):
    from concourse.bass_utils import run_bass_kernel_spmd

    x = np.asarray(x, dtype=np.float32)
    edge_index = np.asarray(edge_index)
    Ws = [np.asarray(W, dtype=np.float64) for W in (W1, W2, W3)]
    bs = [np.asarray(b, dtype=np.float64) for b in (b1, b2, b3)]

    # host-side operator prep
    Wcat = np.concatenate(Ws, axis=1)  # [64, 192]
    wh = np.zeros((128, 6 * C), dtype=np.float16)
    wh[0:64, 0 : 3 * C] = Wcat.astype(np.float16)
    wh[64:128, 3 * C : 6 * C] = Wcat.astype(np.float16)
    ah = np.zeros((3, 128, 128), dtype=np.float16)
    for k in range(3):
        Ak = _dense_adj(edge_index[k])
        blk = Ak.T.astype(np.float16)
        for g in range(5):
            ah[k, g * NNODES : (g + 1) * NNODES, g * NNODES : (g + 1) * NNODES] = blk
    bias = np.zeros(C, dtype=np.float64)
    for b in bs:
        bias += b

    if "nc" not in _PROGRAM_CACHE:
        _PROGRAM_CACHE["nc"] = _build_program()
    nc = _PROGRAM_CACHE["nc"]

    # per-core host-transposed x: [64, 60000] -> [128, 30000]
    x16 = x.reshape(N_CORES, ROWS_LOC, C).astype(np.float16)
    in_maps = []
    for i in range(N_CORES):
        xT = x16[i].T.reshape(C, NTILES // 2, 2, TILE)  # [64, 240, 2, 125]
        xp = np.zeros((128, NTILES // 2, 128), dtype=np.float16)
        xp[0:64, :, :TILE] = xT[:, :, 0]
        xp[64:128, :, :TILE] = xT[:, :, 1]
        in_maps.append(
            {"xh": xp.reshape(128, NTILES // 2 * 128), "wh": wh, "ah": ah}
        )

    res = run_bass_kernel_spmd(nc, in_maps, list(range(N_CORES)), **_RUN_KW)
    _PROGRAM_CACHE["last_result"] = res

    # un-permute: dev[p, m, t, c] = tile (8m+t), row p = 25g+n2, chan c
    outs = []
    for i in range(N_CORES):
        d = res.results[i]["dev"].astype(np.float32)  # [125, 60, 8, 64]
        arr = d.transpose(1, 2, 0, 3).reshape(ROWS_LOC, C)
        outs.append(arr.reshape(BT_LOC, NNODES, C))
    out = np.stack(outs, axis=0)
    if np.any(bias):
        out += bias.astype(np.float32)[None, None, None, :]
    return np.ascontiguousarray(
        out.reshape(B, T, NNODES, C)
    )


# revision 29
# speedup vs baseline: 1.1027x; 1.1027x over previous
"""Trainium2 Bass kernel for a 3-branch GCN layer (sum of three GCNConvs).

Math: out[b,t] = sum_k A_k @ (x[b,t] @ W_k) + b_k, with A_k the normalized
adjacency (self loops) of the k-th tiny 25-node graph shared across (B,T).

Instead of the dense [1600x1600] kron operator (one big GEMM, ~395k PE
row-cycles/core), factor into two chained PE stages with NO on-chip
transposes (host pre-transposes x, which is free):

  stage W:  Y[btn, (k,c)] = X[btn, c'] @ [W1|W2|W3]      (K=64, F=192)
  stage A:  out[btn, c]   = sum_k kron(I5, A_k^T) @ Y_k  (K=125, F=64 x3)

Tiles are 125 rows = 5 (b,t) groups x 25 nodes, padded to 128 so every
stationary is an aligned [128,128] load. Key device-side economics
(measured on TRN2): matmuls carry a ~173ns fixed latency and ~100ns
LDWEIGHTS, so everything is batched fat:

 - W-stage: TWO tiles per matmul: lhsT stacks both tiles' channels in
   K=128, rhs is block-diag [[Wcat,0],[0,Wcat]] -> F=384, 240 matmuls.
 - A-stage: 3 matmuls of F=512 per 8-tile super-group via a 3D rhs over
   the k-major ysb [128, 3, 8t, 64c]; out fills a whole psum bank.
 - PSUM = one 8-bank ring tile: banks 0-5 hold pair-Y accumulations,
   banks 6-7 double-buffer the super-group outputs (accumulation state
   is bank-granular, so exactly one accumulation group per bank).
 - Y is cast fp32->fp16 by per-pair copies alternating ACT/DVE (the only
   PSUM-capable engines), with the (t,k,c)->(k,t,c) transpose folded
   into the copy's AP walk (64-elem contiguous runs).
 - out: fp16 staging copies + gpsimd-queue DMAs; host un-permutes.

Data-parallel over batch: 8 batches (2400 bt rows) per core x 8 cores.
Bias is added on the host (typically zero; np.any fast-path).
"""

import sys

import numpy as np

if "/opt/trn_rl_repo" not in sys.path:
    sys.path.insert(0, "/opt/trn_rl_repo")

B, T, NNODES, C = 64, 300, 25, 64
N_CORES = 8
BT_LOC = (B // N_CORES) * T          # 2400 (b,t) rows per core
ROWS_LOC = BT_LOC * NNODES           # 60000 btn rows per core
TILE = 125                           # 5 bt-groups x 25 nodes
NTILES = ROWS_LOC // TILE            # 480
NGRP = NTILES // 4                   # 120 groups of 4 tiles
NCHUNK = 8                           # x input DMA chunks
HALF = NTILES // 2                   # tiles per partition-half (240)
CHW = NTILES // 2 * 128 // NCHUNK    # x chunk width in elements (3840)
BANKC = 512                          # fp32 elems per psum bank partition-row
OOFF = 256                           # out region offset within a bank

_PROGRAM_CACHE = {}
# extra kwargs for run_bass_kernel_spmd (test harness sets trace=True here)
_RUN_KW = {}


def _dense_adj(edge_index_k: np.ndarray) -> np.ndarray:
    """PyG GCNConv normalized dense adjacency A[dst, src] (float64)."""
    row = edge_index_k[0].astype(np.int64)
    col = edge_index_k[1].astype(np.int64)
    loop = np.arange(NNODES, dtype=np.int64)
    row = np.concatenate([row, loop])
    col = np.concatenate([col, loop])
    deg = np.zeros(NNODES, dtype=np.float64)
    np.add.at(deg, col, 1.0)
    dinv = np.where(deg > 0, 1.0 / np.sqrt(deg), 0.0)
    norm = dinv[row] * dinv[col]
    A = np.zeros((NNODES, NNODES), dtype=np.float64)
    np.add.at(A, (col, row), norm)
    return A


def _build_program():
    import concourse.bass as bass
    import concourse.tile as tile
    from concourse import bacc, mybir

    f32 = mybir.dt.float32
    f16 = mybir.dt.float16

    nc = bacc.Bacc(
        "TRN2", target_bir_lowering=False, debug=False, num_devices=N_CORES
    )
    # host-pretransposed x, tile-pair stacked: [128, 240*128] fp16; rows
    # 0-63 = channels of even tile, 64-127 = channels of odd tile
    xh = nc.dram_tensor(
        "xh", [128, NTILES // 2 * 128], f16, kind="ExternalInput"
    ).ap()
    # block-diagonal [[Wcat, 0], [0, Wcat]] for pair-fused W-matmuls
    wh = nc.dram_tensor("wh", [128, 6 * C], f16, kind="ExternalInput").ap()
    # three zero-padded block-diagonal stationaries kron(I5, A_k^T): [3,128,128]
    ah = nc.dram_tensor("ah", [3, 128, 128], f16, kind="ExternalInput").ap()
    # permuted output: dev[p, m, t, c] = out for btn tile 8m+t, row p, chan c
    dev = nc.dram_tensor(
        "dev", [TILE, NTILES // 8, 8, C], f16, kind="ExternalOutput"
    ).ap()

    DEPTH = 2  # software-pipeline distance, in 8-tile super-groups

    with tile.TileContext(nc) as tc:
        with (
            tc.tile_pool(name="const", bufs=1) as const_pool,
            tc.tile_pool(name="ysb", bufs=6) as ysb_pool,
            tc.tile_pool(name="ostg", bufs=3) as ostg_pool,
            tc.tile_pool(name="ring", bufs=1, space="PSUM") as ring_pool,
        ):
            # the whole of PSUM as one 8-bank ring
            big = ring_pool.tile([128, 8, BANKC], f32, tag="ring", name="ring")

            # constants on the scalar HWDGE queue
            wsb = const_pool.tile([128, 6 * C], f16, tag="wcat")
            nc.scalar.dma_start(wsb[:], wh[:])
            asb = []
            for k in range(3):
                t = const_pool.tile([128, 128], f16, tag=f"a{k}")
                nc.scalar.dma_start(t[:], ah[k])
                asb.append(t)
            
            # x streamed in variable chunks on the sync (SP) queue; the
            # first chunks are small so the PE starts sooner
            sizes = [6, 10, 14] + [30] * 7  # pairs per chunk (240 total)
            xsb = []
            pair_loc = {}
            p0 = 0
            for ci, sz in enumerate(sizes):
                t = const_pool.tile(
                    [128, sz * 128], f16, tag=f"x{ci}", name=f"x{ci}"
                )
                nc.sync.dma_start(t[:], xh[:, p0 * 128 : (p0 + sz) * 128])
                xsb.append(t)
                for j in range(sz):
                    pair_loc[p0 + j] = (ci, j * 128)
                p0 += sz

            def xpair(p):
                # lhsT [128, 128] for tile pair p = tiles (2p, 2p+1)
                ci, off = pair_loc[p]
                return xsb[ci][0:128, off : off + 128]

            ysbs = {}

            def copy(n, dst, src):
                # only DVE and ACT can read PSUM
                if n % 2 == 0:
                    nc.scalar.copy(dst, src)
                else:
                    nc.vector.tensor_copy(dst, src)

            ncopies = [0]
            NSG = NTILES // 8  # 60 super-groups of 8 tiles

            OG = 2  # super-groups per out staging buffer / DMA
            ostg = {"t": None}

            def emit_a_mm(m, k):
                # one accumulating A-matmul, F=512, out = full bank 6 or 7
                ysb = ysbs[m]
                ob = 6 + (m % 2)
                nc.tensor.matmul(
                    big[0:128, ob, 0:BANKC],
                    asb[k][:],
                    ysb[0:128, k, :, :],
                    start=(k == 0), stop=(k == 2),
                )

            def emit_a_out(m):
                ysbs.pop(m)
                ob = 6 + (m % 2)
                go = m % OG
                if go == 0:
                    ostg["t"] = ostg_pool.tile(
                        [TILE, OG, BANKC], f16, tag="ostg", name="ostg"
                    )
                st = ostg["t"]
                copy(ncopies[0], st[0:TILE, go, 0:BANKC], big[0:TILE, ob, 0:BANKC])
                ncopies[0] += 1
                if go == OG - 1:
                    blk = m // OG
                    nc.gpsimd.dma_start(
                        dev[:, OG * blk : OG * (blk + 1), :, :], st[:]
                    )

            def emit_w_mm(p):
                s = p % 6
                nc.tensor.matmul(
                    big[0:128, s, 0 : 6 * C],
                    xpair(p), wsb[:],
                    start=True, stop=True,
                )

            def emit_y_copy(m, h, ysb):
                # drain pair-bank (4m+h)%6: walk (t'', k, c) on the source ==
                # (t, k, c) on the k-major ysb destination; 64-elem contiguous
                # runs keep the engine AP walker fast
                s = (4 * m + h) % 6
                dst = ysb[0:128, :, 2 * h : 2 * h + 2, :].rearrange(
                    "p k t c -> p t k c"
                )
                copy(
                    ncopies[0],
                    dst,
                    big[0:128, s, 0 : 6 * C],
                )
                ncopies[0] += 1

            def emit_sg(m):
                # W-pairs with the previous super-group's fat A-matmuls
                # interleaved between them (v4 ordering)
                ysb = None
                if m < NSG:
                    ysb = ysb_pool.tile([128, 3, 8, C], f16, tag="y", name="y")
                    ysbs[m] = ysb
                for h in range(4):
                    if m < NSG:
                        emit_w_mm(4 * m + h)
                    if m >= DEPTH and h < 3:
                        emit_a_mm(m - DEPTH, h)
                    if m < NSG:
                        emit_y_copy(m, h, ysb)
                if m >= DEPTH:
                    emit_a_out(m - DEPTH)

            for m in range(NSG + DEPTH):
                emit_sg(m)

    nc.compile()
    return nc


def kernel(x, edge_index, W1, W2, W3, b1, b2, b3):
    from concourse.bass_utils import run_bass_kernel_spmd

    x = np.asarray(x, dtype=np.float32)
    edge_index = np.asarray(edge_index)
    Ws = [np.asarray(W, dtype=np.float64) for W in (W1, W2, W3)]
    bs = [np.asarray(b, dtype=np.float64) for b in (b1, b2, b3)]

    # host-side operator prep
    Wcat = np.concatenate(Ws, axis=1)  # [64, 192]
    wh = np.zeros((128, 6 * C), dtype=np.float16)
    wh[0:64, 0 : 3 * C] = Wcat.astype(np.float16)
    wh[64:128, 3 * C : 6 * C] = Wcat.astype(np.float16)
    ah = np.zeros((3, 128, 128), dtype=np.float16)
    for k in range(3):
        Ak = _dense_adj(edge_index[k])
        blk = Ak.T.astype(np.float16)
        for g in range(5):
            ah[k, g * NNODES : (g + 1) * NNODES, g * NNODES : (g + 1) * NNODES] = blk
    bias = np.zeros(C, dtype=np.float64)
    for b in bs:
        bias += b

    if "nc" not in _PROGRAM_CACHE:
        _PROGRAM_CACHE["nc"] = _build_program()
    nc = _PROGRAM_CACHE["nc"]

    # per-core host-transposed x: [64, 60000] -> [128, 30000]
    x16 = x.reshape(N_CORES, ROWS_LOC, C).astype(np.float16)
    in_maps = []
    for i in range(N_CORES):
        xT = x16[i].T.reshape(C, NTILES // 2, 2, TILE)  # [64, 240, 2, 125]
        xp = np.zeros((128, NTILES // 2, 128), dtype=np.float16)
        xp[0:64, :, :TILE] = xT[:, :, 0]
        xp[64:128, :, :TILE] = xT[:, :, 1]
        in_maps.append(
            {"xh": xp.reshape(128, NTILES // 2 * 128), "wh": wh, "ah": ah}
        )

    res = run_bass_kernel_spmd(nc, in_maps, list(range(N_CORES)), **_RUN_KW)
    _PROGRAM_CACHE["last_result"] = res

    # un-permute: dev[p, m, t, c] = tile (8m+t), row p = 25g+n2, chan c
    outs = []
    for i in range(N_CORES):
        d = res.results[i]["dev"].astype(np.float32)  # [125, 60, 8, 64]
        arr = d.transpose(1, 2, 0, 3).reshape(ROWS_LOC, C)
        outs.append(arr.reshape(BT_LOC, NNODES, C))
    out = np.stack(outs, axis=0)
    if np.any(bias):
        out += bias.astype(np.float32)[None, None, None, :]
    return np.ascontiguousarray(
        out.reshape(B, T, NNODES, C)
    )


# revision 31
# speedup vs baseline: 1.1098x; 1.0065x over previous
"""Trainium2 Bass kernel for a 3-branch GCN layer (sum of three GCNConvs).

Math: out[b,t] = sum_k A_k @ (x[b,t] @ W_k) + b_k, with A_k the normalized
adjacency (self loops) of the k-th tiny 25-node graph shared across (B,T).

Instead of the dense [1600x1600] kron operator (one big GEMM, ~395k PE
row-cycles/core), factor into two chained PE stages with NO on-chip
transposes (host pre-transposes x, which is free):

  stage W:  Y[btn, (k,c)] = X[btn, c'] @ [W1|W2|W3]      (pair-fused K=128)
  stage A:  out[btn, c]   = sum_k kron(I5, A_k^T) @ Y_k  (F=512 batched)

Tiles are 125 rows = 5 (b,t) groups x 25 nodes, padded to 128 so every
stationary is an aligned [128,128] load. Key device-side economics
(measured on TRN2): matmuls carry a ~173ns fixed latency and ~100ns
LDWEIGHTS, so everything is batched fat:

 - W-stage: TWO tiles per matmul: lhsT stacks both tiles' channels in
   K=128, rhs is block-diag [[Wcat,0],[0,Wcat]] -> F=384, 240 matmuls.
 - A-stage: 3 matmuls of F=512 per 8-tile super-group via a 3D rhs over
   the k-major ysb [128, 3, 8t, 64c]; out fills a whole psum bank.
 - PSUM = one 8-bank ring tile: banks 0-5 hold pair-Y accumulations,
   banks 6-7 double-buffer the super-group outputs (accumulation state
   is bank-granular, so exactly one accumulation group per bank).
 - Y is cast fp32->fp16 by per-pair copies alternating ACT/DVE (the only
   PSUM-capable engines), with the (t,k,c)->(k,t,c) transpose folded
   into the copy's AP walk (64-elem contiguous runs).
 - out: fp16 staging copies + gpsimd-queue DMAs; host un-permutes.

Data-parallel over batch: 8 batches (2400 bt rows) per core x 8 cores.
Bias is added on the host (typically zero; np.any fast-path).
"""

import sys

import numpy as np

if "/opt/trn_rl_repo" not in sys.path:
    sys.path.insert(0, "/opt/trn_rl_repo")

B, T, NNODES, C = 64, 300, 25, 64
N_CORES = 8
BT_LOC = (B // N_CORES) * T          # 2400 (b,t) rows per core
ROWS_LOC = BT_LOC * NNODES           # 60000 btn rows per core
TILE = 125                           # 5 bt-groups x 25 nodes
NTILES = ROWS_LOC // TILE            # 480
NGRP = NTILES // 4                   # 120 groups of 4 tiles
NCHUNK = 8                           # x input DMA chunks
HALF = NTILES // 2                   # tiles per partition-half (240)
CHW = NTILES // 2 * 128 // NCHUNK    # x chunk width in elements (3840)
BANKC = 512                          # fp32 elems per psum bank partition-row
OOFF = 256                           # out region offset within a bank

_PROGRAM_CACHE = {}
# extra kwargs for run_bass_kernel_spmd (test harness sets trace=True here)
_RUN_KW = {}


def _dense_adj(edge_index_k: np.ndarray) -> np.ndarray:
    """PyG GCNConv normalized dense adjacency A[dst, src] (float64)."""
    row = edge_index_k[0].astype(np.int64)
    col = edge_index_k[1].astype(np.int64)
    loop = np.arange(NNODES, dtype=np.int64)
    row = np.concatenate([row, loop])
    col = np.concatenate([col, loop])
    deg = np.zeros(NNODES, dtype=np.float64)
    np.add.at(deg, col, 1.0)
    dinv = np.where(deg > 0, 1.0 / np.sqrt(deg), 0.0)
    norm = dinv[row] * dinv[col]
    A = np.zeros((NNODES, NNODES), dtype=np.float64)
    np.add.at(A, (col, row), norm)
    return A


def _build_program():
    import concourse.bass as bass
    import concourse.tile as tile
    from concourse import bacc, mybir

    f32 = mybir.dt.float32
    f16 = mybir.dt.float16

    nc = bacc.Bacc(
        "TRN2", target_bir_lowering=False, debug=False, num_devices=N_CORES
    )
    # host-pretransposed x, tile-pair stacked: [128, 240*128] fp16; rows
    # 0-63 = channels of even tile, 64-127 = channels of odd tile
    xh = nc.dram_tensor(
        "xh", [128, NTILES // 2 * 128], f16, kind="ExternalInput"
    ).ap()
    # block-diagonal [[Wcat, 0], [0, Wcat]] for pair-fused W-matmuls
    wh = nc.dram_tensor("wh", [128, 6 * C], f16, kind="ExternalInput").ap()
    # three zero-padded block-diagonal stationaries kron(I5, A_k^T): [3,128,128]
    ah = nc.dram_tensor("ah", [3, 128, 128], f16, kind="ExternalInput").ap()
    # permuted output: dev[p, m, t, c] = out for btn tile 8m+t, row p, chan c
    dev = nc.dram_tensor(
        "dev", [TILE, NTILES // 8, 8, C], f16, kind="ExternalOutput"
    ).ap()

    DEPTH = 2  # software-pipeline distance, in 8-tile super-groups

    with tile.TileContext(nc) as tc:
        with (
            tc.tile_pool(name="const", bufs=1) as const_pool,
            tc.tile_pool(name="ysb", bufs=6) as ysb_pool,
            tc.tile_pool(name="ostg", bufs=3) as ostg_pool,
            tc.tile_pool(name="ring", bufs=1, space="PSUM") as ring_pool,
        ):
            # the whole of PSUM as one 8-bank ring
            big = ring_pool.tile([128, 8, BANKC], f32, tag="ring", name="ring")

            # constants on the scalar HWDGE queue
            wsb = const_pool.tile([128, 6 * C], f16, tag="wcat")
            nc.scalar.dma_start(wsb[:], wh[:])
            asb = []
            for k in range(3):
                t = const_pool.tile([128, 128], f16, tag=f"a{k}")
                nc.scalar.dma_start(t[:], ah[k])
                asb.append(t)
            
            # x streamed in variable chunks on the sync (SP) queue; the
            # first chunks are small so the PE starts sooner
            sizes = [6, 10, 14] + [30] * 7  # pairs per chunk (240 total)
            xsb = []
            pair_loc = {}
            p0 = 0
            for ci, sz in enumerate(sizes):
                t = const_pool.tile(
                    [128, sz * 128], f16, tag=f"x{ci}", name=f"x{ci}"
                )
                nc.sync.dma_start(t[:], xh[:, p0 * 128 : (p0 + sz) * 128])
                xsb.append(t)
                for j in range(sz):
                    pair_loc[p0 + j] = (ci, j * 128)
                p0 += sz

            def xpair(p):
                # lhsT [128, 128] for tile pair p = tiles (2p, 2p+1)
                ci, off = pair_loc[p]
                return xsb[ci][0:128, off : off + 128]

            ysbs = {}

            def copy(n, dst, src):
                # only DVE and ACT can read PSUM
                if n % 2 == 0:
                    nc.scalar.copy(dst, src)
                else:
                    nc.vector.tensor_copy(dst, src)

            ncopies = [0]
            NSG = NTILES // 8  # 60 super-groups of 8 tiles

            OG = 2  # super-groups per out staging buffer / DMA
            ostg = {"t": None}

            def emit_a_mm(m, k):
                # one accumulating A-matmul, F=512, out = full bank 6 or 7
                ysb = ysbs[m]
                ob = 6 + (m % 2)
                nc.tensor.matmul(
                    big[0:128, ob, 0:BANKC],
                    asb[k][:],
                    ysb[0:128, k, :, :],
                    start=(k == 0), stop=(k == 2),
                )

            def emit_a_out(m):
                ysbs.pop(m)
                ob = 6 + (m % 2)
                go = m % OG
                if go == 0:
                    ostg["t"] = ostg_pool.tile(
                        [TILE, OG, BANKC], f16, tag="ostg", name="ostg"
                    )
                st = ostg["t"]
                copy(ncopies[0], st[0:TILE, go, 0:BANKC], big[0:TILE, ob, 0:BANKC])
                ncopies[0] += 1
                if go == OG - 1:
                    blk = m // OG
                    nc.gpsimd.dma_start(
                        dev[:, OG * blk : OG * (blk + 1), :, :], st[:]
                    )

            def emit_w_mm(p):
                s = p % 6
                nc.tensor.matmul(
                    big[0:128, s, 0 : 6 * C],
                    xpair(p), wsb[:],
                    start=True, stop=True,
                )

            def emit_y_copy(m, u, ysb):
                # one copy drains TWO pair-banks: walk (bank, t'', k, c) on
                # the source == (t, k, c) on the k-major ysb destination
                s = (4 * m + 2 * u) % 6
                dst = ysb[0:128, :, 4 * u : 4 * u + 4, :].rearrange(
                    "p k t c -> p t k c"
                )
                copy(
                    ncopies[0],
                    dst,
                    big[0:128, s : s + 2, 0 : 6 * C],
                )
                ncopies[0] += 1

            def emit_sg(m):
                # W-pairs with the previous super-group's fat A-matmuls
                # interleaved between them (v4 ordering)
                ysb = None
                if m < NSG:
                    ysb = ysb_pool.tile([128, 3, 8, C], f16, tag="y", name="y")
                    ysbs[m] = ysb
                for h in range(4):
                    if m < NSG:
                        emit_w_mm(4 * m + h)
                    if m >= DEPTH and h < 3:
                        emit_a_mm(m - DEPTH, h)
                    if m < NSG and h % 2 == 1:
                        emit_y_copy(m, h // 2, ysb)
                if m >= DEPTH:
                    emit_a_out(m - DEPTH)

            for m in range(NSG + DEPTH):
                emit_sg(m)

    nc.compile()
    return nc


def kernel(x, edge_index, W1, W2, W3, b1, b2, b3):
    from concourse.bass_utils import run_bass_kernel_spmd

    x = np.asarray(x, dtype=np.float32)
    edge_index = np.asarray(edge_index)
    Ws = [np.asarray(W, dtype=np.float64) for W in (W1, W2, W3)]
    bs = [np.asarray(b, dtype=np.float64) for b in (b1, b2, b3)]

    # host-side operator prep
    Wcat = np.concatenate(Ws, axis=1)  # [64, 192]
    wh = np.zeros((128, 6 * C), dtype=np.float16)
    wh[0:64, 0 : 3 * C] = Wcat.astype(np.float16)
    wh[64:128, 3 * C : 6 * C] = Wcat.astype(np.float16)
    ah = np.zeros((3, 128, 128), dtype=np.float16)
    for k in range(3):
        Ak = _dense_adj(edge_index[k])
        blk = Ak.T.astype(np.float16)
        for g in range(5):
            ah[k, g * NNODES : (g + 1) * NNODES, g * NNODES : (g + 1) * NNODES] = blk
    bias = np.zeros(C, dtype=np.float64)
    for b in bs:
        bias += b

    if "nc" not in _PROGRAM_CACHE:
        _PROGRAM_CACHE["nc"] = _build_program()
    nc = _PROGRAM_CACHE["nc"]

    # per-core host-transposed x: [64, 60000] -> [128, 30000]
    x16 = x.reshape(N_CORES, ROWS_LOC, C).astype(np.float16)
    in_maps = []
    for i in range(N_CORES):
        xT = x16[i].T.reshape(C, NTILES // 2, 2, TILE)  # [64, 240, 2, 125]
        xp = np.zeros((128, NTILES // 2, 128), dtype=np.float16)
        xp[0:64, :, :TILE] = xT[:, :, 0]
        xp[64:128, :, :TILE] = xT[:, :, 1]
        in_maps.append(
            {"xh": xp.reshape(128, NTILES // 2 * 128), "wh": wh, "ah": ah}
        )

    res = run_bass_kernel_spmd(nc, in_maps, list(range(N_CORES)), **_RUN_KW)
    _PROGRAM_CACHE["last_result"] = res

    # un-permute: dev[p, m, t, c] = tile (8m+t), row p = 25g+n2, chan c
    outs = []
    for i in range(N_CORES):
        d = res.results[i]["dev"].astype(np.float32)  # [125, 60, 8, 64]
        arr = d.transpose(1, 2, 0, 3).reshape(ROWS_LOC, C)
        outs.append(arr.reshape(BT_LOC, NNODES, C))
    out = np.stack(outs, axis=0)
    if np.any(bias):
        out += bias.astype(np.float32)[None, None, None, :]
    return np.ascontiguousarray(
        out.reshape(B, T, NNODES, C)
    )


# revision 32
# speedup vs baseline: 1.1633x; 1.0481x over previous
"""Trainium2 Bass kernel for a 3-branch GCN layer (sum of three GCNConvs).

Math: out[b,t] = sum_k A_k @ (x[b,t] @ W_k) + b_k, with A_k the normalized
adjacency (self loops) of the k-th tiny 25-node graph shared across (B,T).

Instead of the dense [1600x1600] kron operator (one big GEMM, ~395k PE
row-cycles/core), factor into two chained PE stages with NO on-chip
transposes (host pre-transposes x, which is free):

  stage W:  Y[btn, (k,c)] = X[btn, c'] @ [W1|W2|W3]      (pair-fused K=128)
  stage A:  out[btn, c]   = sum_k kron(I5, A_k^T) @ Y_k  (F=512 batched)

Tiles are 125 rows = 5 (b,t) groups x 25 nodes, padded to 128 so every
stationary is an aligned [128,128] load. Key device-side economics
(measured on TRN2): matmuls carry a ~173ns fixed latency and ~100ns
LDWEIGHTS, so everything is batched fat:

 - W-stage: TWO tiles per matmul: lhsT stacks both tiles' channels in
   K=128, rhs is block-diag [[Wcat,0],[0,Wcat]] -> F=384, 240 matmuls.
 - A-stage: 3 matmuls of F=512 per 8-tile super-group via a 3D rhs over
   the k-major ysb [128, 3, 8t, 64c]; out fills a whole psum bank.
 - PSUM = one 8-bank ring tile: banks 0-5 hold pair-Y accumulations,
   banks 6-7 double-buffer the super-group outputs (accumulation state
   is bank-granular, so exactly one accumulation group per bank).
 - Y is cast fp32->fp16 by per-pair copies alternating ACT/DVE (the only
   PSUM-capable engines), with the (t,k,c)->(k,t,c) transpose folded
   into the copy's AP walk (64-elem contiguous runs).
 - out: fp16 staging copies + gpsimd-queue DMAs; host un-permutes.

Data-parallel over batch: 8 batches (2400 bt rows) per core x 8 cores.
Bias is added on the host (typically zero; np.any fast-path).
"""

import sys

import numpy as np

if "/opt/trn_rl_repo" not in sys.path:
    sys.path.insert(0, "/opt/trn_rl_repo")

B, T, NNODES, C = 64, 300, 25, 64
N_CORES = 8
BT_LOC = (B // N_CORES) * T          # 2400 (b,t) rows per core
ROWS_LOC = BT_LOC * NNODES           # 60000 btn rows per core
TILE = 125                           # 5 bt-groups x 25 nodes
NTILES = ROWS_LOC // TILE            # 480
NGRP = NTILES // 4                   # 120 groups of 4 tiles
NCHUNK = 8                           # x input DMA chunks
HALF = NTILES // 2                   # tiles per partition-half (240)
CHW = NTILES // 2 * 128 // NCHUNK    # x chunk width in elements (3840)
BANKC = 512                          # fp32 elems per psum bank partition-row
OOFF = 256                           # out region offset within a bank

_PROGRAM_CACHE = {}
# extra kwargs for run_bass_kernel_spmd (test harness sets trace=True here)
_RUN_KW = {}


def _dense_adj(edge_index_k: np.ndarray) -> np.ndarray:
    """PyG GCNConv normalized dense adjacency A[dst, src] (float64)."""
    row = edge_index_k[0].astype(np.int64)
    col = edge_index_k[1].astype(np.int64)
    loop = np.arange(NNODES, dtype=np.int64)
    row = np.concatenate([row, loop])
    col = np.concatenate([col, loop])
    deg = np.zeros(NNODES, dtype=np.float64)
    np.add.at(deg, col, 1.0)
    dinv = np.where(deg > 0, 1.0 / np.sqrt(deg), 0.0)
    norm = dinv[row] * dinv[col]
    A = np.zeros((NNODES, NNODES), dtype=np.float64)
    np.add.at(A, (col, row), norm)
    return A


def _build_program():
    import concourse.bass as bass
    import concourse.tile as tile
    from concourse import bacc, mybir

    f32 = mybir.dt.float32
    f16 = mybir.dt.float16

    nc = bacc.Bacc(
        "TRN2", target_bir_lowering=False, debug=False, num_devices=N_CORES
    )
    # host-pretransposed x, tile-pair stacked: [128, 240*128] fp16; rows
    # 0-63 = channels of even tile, 64-127 = channels of odd tile
    xh = nc.dram_tensor(
        "xh", [128, NTILES // 2 * 128], f16, kind="ExternalInput"
    ).ap()
    # block-diagonal [[Wcat, 0], [0, Wcat]] for pair-fused W-matmuls
    wh = nc.dram_tensor("wh", [128, 6 * C], f16, kind="ExternalInput").ap()
    # three zero-padded block-diagonal stationaries kron(I5, A_k^T): [3,128,128]
    ah = nc.dram_tensor("ah", [3, 128, 128], f16, kind="ExternalInput").ap()
    # permuted output: dev[p, m, t, c] = out for btn tile 8m+t, row p, chan c
    dev = nc.dram_tensor(
        "dev", [TILE, NTILES // 8, 8, C], f16, kind="ExternalOutput"
    ).ap()

    DEPTH = 3  # software-pipeline distance, in 8-tile super-groups

    with tile.TileContext(nc) as tc:
        with (
            tc.tile_pool(name="const", bufs=1) as const_pool,
            tc.tile_pool(name="ysb", bufs=8) as ysb_pool,
            tc.tile_pool(name="ostg", bufs=3) as ostg_pool,
            tc.tile_pool(name="ring", bufs=1, space="PSUM") as ring_pool,
        ):
            # the whole of PSUM as one 8-bank ring
            big = ring_pool.tile([128, 8, BANKC], f32, tag="ring", name="ring")

            # constants on the scalar HWDGE queue
            wsb = const_pool.tile([128, 6 * C], f16, tag="wcat")
            nc.scalar.dma_start(wsb[:], wh[:])
            asb = []
            for k in range(3):
                t = const_pool.tile([128, 128], f16, tag=f"a{k}")
                nc.scalar.dma_start(t[:], ah[k])
                asb.append(t)
            
            # x streamed in variable chunks on the sync (SP) queue; the
            # first chunks are small so the PE starts sooner
            sizes = [6, 10, 14] + [30] * 7  # pairs per chunk (240 total)
            xsb = []
            pair_loc = {}
            p0 = 0
            for ci, sz in enumerate(sizes):
                t = const_pool.tile(
                    [128, sz * 128], f16, tag=f"x{ci}", name=f"x{ci}"
                )
                nc.sync.dma_start(t[:], xh[:, p0 * 128 : (p0 + sz) * 128])
                xsb.append(t)
                for j in range(sz):
                    pair_loc[p0 + j] = (ci, j * 128)
                p0 += sz

            def xpair(p):
                # lhsT [128, 128] for tile pair p = tiles (2p, 2p+1)
                ci, off = pair_loc[p]
                return xsb[ci][0:128, off : off + 128]

            ysbs = {}

            def copy(n, dst, src):
                # only DVE and ACT can read PSUM
                if n % 2 == 0:
                    nc.scalar.copy(dst, src)
                else:
                    nc.vector.tensor_copy(dst, src)

            ncopies = [0]
            NSG = NTILES // 8  # 60 super-groups of 8 tiles

            OG = 4  # super-groups per out staging buffer / DMA
            ostg = {"t": None}

            def emit_a_mm(m, k):
                # one accumulating A-matmul, F=512, out = full bank 6 or 7
                ysb = ysbs[m]
                ob = 6 + (m % 2)
                nc.tensor.matmul(
                    big[0:128, ob, 0:BANKC],
                    asb[k][:],
                    ysb[0:128, k, :, :],
                    start=(k == 0), stop=(k == 2),
                )

            def emit_a_out(m):
                ysbs.pop(m)
                ob = 6 + (m % 2)
                go = m % OG
                if go == 0:
                    ostg["t"] = ostg_pool.tile(
                        [TILE, OG, BANKC], f16, tag="ostg", name="ostg"
                    )
                st = ostg["t"]
                copy(ncopies[0], st[0:TILE, go, 0:BANKC], big[0:TILE, ob, 0:BANKC])
                ncopies[0] += 1
                if go == OG - 1:
                    blk = m // OG
                    nc.gpsimd.dma_start(
                        dev[:, OG * blk : OG * (blk + 1), :, :], st[:]
                    )

            def emit_w_mm(p):
                s = p % 6
                nc.tensor.matmul(
                    big[0:128, s, 0 : 6 * C],
                    xpair(p), wsb[:],
                    start=True, stop=True,
                )

            def emit_y_copy(m, u, ysb):
                # one copy drains TWO pair-banks: walk (bank, t'', k, c) on
                # the source == (t, k, c) on the k-major ysb destination
                s = (4 * m + 2 * u) % 6
                dst = ysb[0:128, :, 4 * u : 4 * u + 4, :].rearrange(
                    "p k t c -> p t k c"
                )
                copy(
                    ncopies[0],
                    dst,
                    big[0:128, s : s + 2, 0 : 6 * C],
                )
                ncopies[0] += 1

            def emit_sg(m):
                # W-pairs with the previous super-group's fat A-matmuls
                # interleaved between them (v4 ordering)
                ysb = None
                if m < NSG:
                    ysb = ysb_pool.tile([128, 3, 8, C], f16, tag="y", name="y")
                    ysbs[m] = ysb
                for h in range(4):
                    if m < NSG:
                        emit_w_mm(4 * m + h)
                    if m >= DEPTH and h < 3:
                        emit_a_mm(m - DEPTH, h)
                    if m < NSG and h % 2 == 1:
                        emit_y_copy(m, h // 2, ysb)
                if m >= DEPTH:
                    emit_a_out(m - DEPTH)

            for m in range(NSG + DEPTH):
                emit_sg(m)

    nc.compile()
    return nc


def kernel(x, edge_index, W1, W2, W3, b1, b2, b3):
    from concourse.bass_utils import run_bass_kernel_spmd

    x = np.asarray(x, dtype=np.float32)
    edge_index = np.asarray(edge_index)
    Ws = [np.asarray(W, dtype=np.float64) for W in (W1, W2, W3)]
    bs = [np.asarray(b, dtype=np.float64) for b in (b1, b2, b3)]

    # host-side operator prep
    Wcat = np.concatenate(Ws, axis=1)  # [64, 192]
    wh = np.zeros((128, 6 * C), dtype=np.float16)
    wh[0:64, 0 : 3 * C] = Wcat.astype(np.float16)
    wh[64:128, 3 * C : 6 * C] = Wcat.astype(np.float16)
    ah = np.zeros((3, 128, 128), dtype=np.float16)
    for k in range(3):
        Ak = _dense_adj(edge_index[k])
        blk = Ak.T.astype(np.float16)
        for g in range(5):
            ah[k, g * NNODES : (g + 1) * NNODES, g * NNODES : (g + 1) * NNODES] = blk
    bias = np.zeros(C, dtype=np.float64)
    for b in bs:
        bias += b

    if "nc" not in _PROGRAM_CACHE:
        _PROGRAM_CACHE["nc"] = _build_program()
    nc = _PROGRAM_CACHE["nc"]

    # per-core host-transposed x: [64, 60000] -> [128, 30000]
    x16 = x.reshape(N_CORES, ROWS_LOC, C).astype(np.float16)
    in_maps = []
    for i in range(N_CORES):
        xT = x16[i].T.reshape(C, NTILES // 2, 2, TILE)  # [64, 240, 2, 125]
        xp = np.zeros((128, NTILES // 2, 128), dtype=np.float16)
        xp[0:64, :, :TILE] = xT[:, :, 0]
        xp[64:128, :, :TILE] = xT[:, :, 1]
        in_maps.append(
            {"xh": xp.reshape(128, NTILES // 2 * 128), "wh": wh, "ah": ah}
        )

    res = run_bass_kernel_spmd(nc, in_maps, list(range(N_CORES)), **_RUN_KW)
    _PROGRAM_CACHE["last_result"] = res

    # un-permute: dev[p, m, t, c] = tile (8m+t), row p = 25g+n2, chan c
    outs = []
    for i in range(N_CORES):
        d = res.results[i]["dev"].astype(np.float32)  # [125, 60, 8, 64]
        arr = d.transpose(1, 2, 0, 3).reshape(ROWS_LOC, C)
        outs.append(arr.reshape(BT_LOC, NNODES, C))
    out = np.stack(outs, axis=0)
    if np.any(bias):
        out += bias.astype(np.float32)[None, None, None, :]
    return np.ascontiguousarray(
        out.reshape(B, T, NNODES, C)
    )


# revision 33
# speedup vs baseline: 1.1716x; 1.0071x over previous
"""Trainium2 Bass kernel for a 3-branch GCN layer (sum of three GCNConvs).

Math: out[b,t] = sum_k A_k @ (x[b,t] @ W_k) + b_k, with A_k the normalized
adjacency (self loops) of the k-th tiny 25-node graph shared across (B,T).

Instead of the dense [1600x1600] kron operator (one big GEMM, ~395k PE
row-cycles/core), factor into two chained PE stages with NO on-chip
transposes (host pre-transposes x, which is free):

  stage W:  Y[btn, (k,c)] = X[btn, c'] @ [W1|W2|W3]      (pair-fused K=128)
  stage A:  out[btn, c]   = sum_k kron(I5, A_k^T) @ Y_k  (F=512 batched)

Tiles are 125 rows = 5 (b,t) groups x 25 nodes, padded to 128 so every
stationary is an aligned [128,128] load. Key device-side economics
(measured on TRN2): matmuls carry a ~173ns fixed latency and ~100ns
LDWEIGHTS, so everything is batched fat:

 - W-stage: TWO tiles per matmul: lhsT stacks both tiles' channels in
   K=128, rhs is block-diag [[Wcat,0],[0,Wcat]] -> F=384, 240 matmuls.
 - A-stage: 3 matmuls of F=512 per 8-tile super-group via a 3D rhs over
   the k-major ysb [128, 3, 8t, 64c]; out fills a whole psum bank.
 - PSUM = one 8-bank ring tile: banks 0-5 hold pair-Y accumulations,
   banks 6-7 double-buffer the super-group outputs (accumulation state
   is bank-granular, so exactly one accumulation group per bank).
 - Y is cast fp32->fp16 by per-pair copies alternating ACT/DVE (the only
   PSUM-capable engines), with the (t,k,c)->(k,t,c) transpose folded
   into the copy's AP walk (64-elem contiguous runs).
 - out: fp16 staging copies + gpsimd-queue DMAs; host un-permutes.

Data-parallel over batch: 8 batches (2400 bt rows) per core x 8 cores.
Bias is added on the host (typically zero; np.any fast-path).
"""

import sys

import numpy as np

if "/opt/trn_rl_repo" not in sys.path:
    sys.path.insert(0, "/opt/trn_rl_repo")

B, T, NNODES, C = 64, 300, 25, 64
N_CORES = 8
BT_LOC = (B // N_CORES) * T          # 2400 (b,t) rows per core
ROWS_LOC = BT_LOC * NNODES           # 60000 btn rows per core
TILE = 125                           # 5 bt-groups x 25 nodes
NTILES = ROWS_LOC // TILE            # 480
NGRP = NTILES // 4                   # 120 groups of 4 tiles
NCHUNK = 8                           # x input DMA chunks
HALF = NTILES // 2                   # tiles per partition-half (240)
CHW = NTILES // 2 * 128 // NCHUNK    # x chunk width in elements (3840)
BANKC = 512                          # fp32 elems per psum bank partition-row
OOFF = 256                           # out region offset within a bank

_PROGRAM_CACHE = {}
# extra kwargs for run_bass_kernel_spmd (test harness sets trace=True here)
_RUN_KW = {}


def _dense_adj(edge_index_k: np.ndarray) -> np.ndarray:
    """PyG GCNConv normalized dense adjacency A[dst, src] (float64)."""
    row = edge_index_k[0].astype(np.int64)
    col = edge_index_k[1].astype(np.int64)
    loop = np.arange(NNODES, dtype=np.int64)
    row = np.concatenate([row, loop])
    col = np.concatenate([col, loop])
    deg = np.zeros(NNODES, dtype=np.float64)
    np.add.at(deg, col, 1.0)
    dinv = np.where(deg > 0, 1.0 / np.sqrt(deg), 0.0)
    norm = dinv[row] * dinv[col]
    A = np.zeros((NNODES, NNODES), dtype=np.float64)
    np.add.at(A, (col, row), norm)
    return A


def _build_program():
    import concourse.bass as bass
    import concourse.tile as tile
    from concourse import bacc, mybir

    f32 = mybir.dt.float32
    f16 = mybir.dt.float16

    nc = bacc.Bacc(
        "TRN2", target_bir_lowering=False, debug=False, num_devices=N_CORES
    )
    # host-pretransposed x, tile-pair stacked: [128, 240*128] fp16; rows
    # 0-63 = channels of even tile, 64-127 = channels of odd tile
    xh = nc.dram_tensor(
        "xh", [128, NTILES // 2 * 128], f16, kind="ExternalInput"
    ).ap()
    # block-diagonal [[Wcat, 0], [0, Wcat]] for pair-fused W-matmuls
    wh = nc.dram_tensor("wh", [128, 6 * C], f16, kind="ExternalInput").ap()
    # three zero-padded block-diagonal stationaries kron(I5, A_k^T): [3,128,128]
    ah = nc.dram_tensor("ah", [3, 128, 128], f16, kind="ExternalInput").ap()
    # permuted output: dev[p, m, t, c] = out for btn tile 8m+t, row p, chan c
    dev = nc.dram_tensor(
        "dev", [TILE, NTILES // 8, 8, C], f16, kind="ExternalOutput"
    ).ap()

    DEPTH = 4  # software-pipeline distance, in 8-tile super-groups

    with tile.TileContext(nc) as tc:
        with (
            tc.tile_pool(name="const", bufs=1) as const_pool,
            tc.tile_pool(name="ysb", bufs=10) as ysb_pool,
            tc.tile_pool(name="ostg", bufs=3) as ostg_pool,
            tc.tile_pool(name="ring", bufs=1, space="PSUM") as ring_pool,
        ):
            # the whole of PSUM as one 8-bank ring
            big = ring_pool.tile([128, 8, BANKC], f32, tag="ring", name="ring")

            # constants on the scalar HWDGE queue
            wsb = const_pool.tile([128, 6 * C], f16, tag="wcat")
            nc.scalar.dma_start(wsb[:], wh[:])
            asb = []
            for k in range(3):
                t = const_pool.tile([128, 128], f16, tag=f"a{k}")
                nc.scalar.dma_start(t[:], ah[k])
                asb.append(t)
            
            # x streamed in variable chunks on the sync (SP) queue; the
            # first chunks are small so the PE starts sooner
            sizes = [6, 10, 14] + [30] * 7  # pairs per chunk (240 total)
            xsb = []
            pair_loc = {}
            p0 = 0
            for ci, sz in enumerate(sizes):
                t = const_pool.tile(
                    [128, sz * 128], f16, tag=f"x{ci}", name=f"x{ci}"
                )
                nc.sync.dma_start(t[:], xh[:, p0 * 128 : (p0 + sz) * 128])
                xsb.append(t)
                for j in range(sz):
                    pair_loc[p0 + j] = (ci, j * 128)
                p0 += sz

            def xpair(p):
                # lhsT [128, 128] for tile pair p = tiles (2p, 2p+1)
                ci, off = pair_loc[p]
                return xsb[ci][0:128, off : off + 128]

            ysbs = {}

            def copy(n, dst, src):
                # only DVE and ACT can read PSUM
                if n % 2 == 0:
                    nc.scalar.copy(dst, src)
                else:
                    nc.vector.tensor_copy(dst, src)

            ncopies = [0]
            NSG = NTILES // 8  # 60 super-groups of 8 tiles

            OG = 4  # super-groups per out staging buffer / DMA
            ostg = {"t": None}

            def emit_a_mm(m, k):
                # one accumulating A-matmul, F=512, out = full bank 6 or 7
                ysb = ysbs[m]
                ob = 6 + (m % 2)
                nc.tensor.matmul(
                    big[0:128, ob, 0:BANKC],
                    asb[k][:],
                    ysb[0:128, k, :, :],
                    start=(k == 0), stop=(k == 2),
                )

            def emit_a_out(m):
                ysbs.pop(m)
                ob = 6 + (m % 2)
                go = m % OG
                if go == 0:
                    ostg["t"] = ostg_pool.tile(
                        [TILE, OG, BANKC], f16, tag="ostg", name="ostg"
                    )
                st = ostg["t"]
                copy(ncopies[0], st[0:TILE, go, 0:BANKC], big[0:TILE, ob, 0:BANKC])
                ncopies[0] += 1
                if go == OG - 1:
                    blk = m // OG
                    nc.gpsimd.dma_start(
                        dev[:, OG * blk : OG * (blk + 1), :, :], st[:]
                    )

            def emit_w_mm(p):
                s = p % 6
                nc.tensor.matmul(
                    big[0:128, s, 0 : 6 * C],
                    xpair(p), wsb[:],
                    start=True, stop=True,
                )

            def emit_y_copy(m, u, ysb):
                # one copy drains TWO pair-banks: walk (bank, t'', k, c) on
                # the source == (t, k, c) on the k-major ysb destination
                s = (4 * m + 2 * u) % 6
                dst = ysb[0:128, :, 4 * u : 4 * u + 4, :].rearrange(
                    "p k t c -> p t k c"
                )
                copy(
                    ncopies[0],
                    dst,
                    big[0:128, s : s + 2, 0 : 6 * C],
                )
                ncopies[0] += 1

            def emit_sg(m):
                # W-pairs with the previous super-group's fat A-matmuls
                # interleaved between them (v4 ordering)
                ysb = None
                if m < NSG:
                    ysb = ysb_pool.tile([128, 3, 8, C], f16, tag="y", name="y")
                    ysbs[m] = ysb
                for h in range(4):
                    if m < NSG:
                        emit_w_mm(4 * m + h)
                    if m >= DEPTH and h < 3:
                        emit_a_mm(m - DEPTH, h)
                    if m < NSG and h % 2 == 1:
                        emit_y_copy(m, h // 2, ysb)
                if m >= DEPTH:
                    emit_a_out(m - DEPTH)

            for m in range(NSG + DEPTH):
                emit_sg(m)

    nc.compile()
    return nc


def kernel(x, edge_index, W1, W2, W3, b1, b2, b3):
    from concourse.bass_utils import run_bass_kernel_spmd

    x = np.asarray(x, dtype=np.float32)
    edge_index = np.asarray(edge_index)
    Ws = [np.asarray(W, dtype=np.float64) for W in (W1, W2, W3)]
    bs = [np.asarray(b, dtype=np.float64) for b in (b1, b2, b3)]

    # host-side operator prep
    Wcat = np.concatenate(Ws, axis=1)  # [64, 192]
    wh = np.zeros((128, 6 * C), dtype=np.float16)
    wh[0:64, 0 : 3 * C] = Wcat.astype(np.float16)
    wh[64:128, 3 * C : 6 * C] = Wcat.astype(np.float16)
    ah = np.zeros((3, 128, 128), dtype=np.float16)
    for k in range(3):
        Ak = _dense_adj(edge_index[k])
        blk = Ak.T.astype(np.float16)
        for g in range(5):
            ah[k, g * NNODES : (g + 1) * NNODES, g * NNODES : (g + 1) * NNODES] = blk
    bias = np.zeros(C, dtype=np.float64)
    for b in bs:
        bias += b

    if "nc" not in _PROGRAM_CACHE:
        _PROGRAM_CACHE["nc"] = _build_program()
    nc = _PROGRAM_CACHE["nc"]

    # per-core host-transposed x: [64, 60000] -> [128, 30000]
    x16 = x.reshape(N_CORES, ROWS_LOC, C).astype(np.float16)
    in_maps = []
    for i in range(N_CORES):
        xT = x16[i].T.reshape(C, NTILES // 2, 2, TILE)  # [64, 240, 2, 125]
        xp = np.zeros((128, NTILES // 2, 128), dtype=np.float16)
        xp[0:64, :, :TILE] = xT[:, :, 0]
        xp[64:128, :, :TILE] = xT[:, :, 1]
        in_maps.append(
            {"xh": xp.reshape(128, NTILES // 2 * 128), "wh": wh, "ah": ah}
        )

    res = run_bass_kernel_spmd(nc, in_maps, list(range(N_CORES)), **_RUN_KW)
    _PROGRAM_CACHE["last_result"] = res

    # un-permute: dev[p, m, t, c] = tile (8m+t), row p = 25g+n2, chan c
    outs = []
    for i in range(N_CORES):
        d = res.results[i]["dev"].astype(np.float32)  # [125, 60, 8, 64]
        arr = d.transpose(1, 2, 0, 3).reshape(ROWS_LOC, C)
        outs.append(arr.reshape(BT_LOC, NNODES, C))
    out = np.stack(outs, axis=0)
    if np.any(bias):
        out += bias.astype(np.float32)[None, None, None, :]
    return np.ascontiguousarray(
        out.reshape(B, T, NNODES, C)
    )
